# revision 43
# baseline (speedup 1.0000x reference)
"""Llama attention layer (B=2, S=2048, H=4096, 32 q heads / 8 kv heads, HD=128)
on 8 Trainium2 NeuronCores, tensor-parallel over heads.

Host->device traffic is the wall-clock bottleneck (axon-tunneled cores), so
inputs are sharded to minimize bytes over the tunnel:
  - hidden_states: each core receives a distinct 512-token slice of hsT
    (4MB/core instead of a full 32MB replica); an on-device AllGather
    reconstructs the full [H, T] activation in shared DRAM before the
    projections.
  - cos/sin RoPE tables: same 512-token sharding + AllGather (0.25MB/core).
  - weights: per-core head shards (Wq/Wo 512 cols, Wk/Wv 128 cols), bf16.
  - donated output buffers are created device-side (jnp.zeros under jit),
    not transferred.

Per core c (SPMD, identical program, different data):
  - weights: Wq[:, 512c:512c+512], Wk/Wv[:, 128c:128c+128], Wo[:, 512c:512c+512]
  - qT/kT/vT projections (weights stationary, transposed outputs), RoPE on DVE
    with host-precomputed cos/sin tables
  - attention computed transposed (scoresT = [k-tokens, q-tokens]) so exp'd
    score tiles feed the PV matmul as lhsT with no transposes; softmax
    normalization deferred via a ones-column appended to V (row sums land
    per-partition); causal = only lower blocks + masked diagonal tile
  - AllGather of per-core attention outputs in 4 token chunks (overlapped with
    compute), then column-sharded o_proj producing the transposed output slice
All matmuls bf16 with fp32 PSUM accumulation.
"""

import sys

sys.path.insert(0, "/opt/trn_rl_repo")

import numpy as np
import ml_dtypes

B, S, H = 2, 2048, 4096
NQ, NKV, HD = 32, 8, 128
T = B * S  # 4096 global tokens, j = b*S + s
NCORES = 8
HQ = NQ // NCORES  # 4 q heads per core
DQ = HQ * HD  # 512 q dims per core
ROPE_THETA = 10000.0
SM_SCALE = 1.0 / float(np.sqrt(HD))

NB_S = S // 128  # 16 token blocks per batch
KCH = H // 128  # 32 contraction chunks
TC_W = 512  # token chunk width in attention/o_proj phases
TSH = T // NCORES  # 512-token hs shard per core
NSC = S // TSH  # 4 hs shards per batch
NCHUNK = 4  # allgather token chunks (attention output)
CH_W = T // NCHUNK  # 1024 tokens per gather chunk
OG = 128  # output int8 scale group width (tokens)
NOG = T // OG  # scale groups per output row
# int8 hidden_states transfer was tried and reverted: quantization noise on
# q/k scores is amplified by softmax (measured rel err 1.96e-2 vs the 2e-2
# gate) for only ~0.12s net saving. Keep activations bf16 on the wire.
HS_INT8 = False

_state = {}


def _build():
    import concourse.bass as bass
    import concourse.mybir as mybir
    import concourse.tile as tile
    from concourse import bacc
    from concourse.masks import make_identity, make_upper_triangular

    f32 = mybir.dt.float32
    bf16 = mybir.dt.bfloat16

    nc = bacc.Bacc("TRN2", target_bir_lowering=False, debug=False,
                   num_devices=NCORES)

    i8 = mybir.dt.int8
    hs_dt = i8 if HS_INT8 else bf16
    hs_sh = nc.dram_tensor("hs_sh", [H, TSH], hs_dt, kind="ExternalInput").ap()
    if HS_INT8:
        # per-row dequant scales, host-transposed to [ki, ko] so SBUF loads
        # are contiguous per partition
        hsc_sh = nc.dram_tensor("hsc_sh", [128, KCH], f32,
                                kind="ExternalInput").ap()
    wq = nc.dram_tensor("wq", [H, DQ], bf16, kind="ExternalInput").ap()
    wk = nc.dram_tensor("wk", [H, HD], bf16, kind="ExternalInput").ap()
    wv = nc.dram_tensor("wv", [H, HD], bf16, kind="ExternalInput").ap()
    wo = nc.dram_tensor("wo", [H, DQ], bf16, kind="ExternalInput").ap()
    cs_sh = nc.dram_tensor("cs_sh", [2 * HD, TSH], bf16,
                           kind="ExternalInput").ap()
    # output wire format: int8 with per-(row, OG-token-group) absmax scales
    # (halves D2H bytes; host dequantizes)
    outT_i8 = nc.dram_tensor("outT_i8", [DQ, T], i8, kind="ExternalOutput").ap()
    oscale = nc.dram_tensor("oscale", [DQ, NOG], f32,
                            kind="ExternalOutput").ap()

    # gathered activations / rope tables (written once by the start-of-kernel
    # AllGathers, read by projections). Collectives cannot read IO tensors, so
    # the input shards are staged through internal DRAM first.
    hs_stg = nc.dram_tensor("hs_stg", [H, TSH], hs_dt).ap()
    cs_stg = nc.dram_tensor("cs_stg", [2 * HD, TSH], bf16).ap()
    g_hs = nc.dram_tensor("g_hs", [NCORES * H, TSH], hs_dt,
                          addr_space="Shared").ap()
    g_cs = nc.dram_tensor("g_cs", [NCORES * 2 * HD, TSH], bf16,
                          addr_space="Shared").ap()
    if HS_INT8:
        hsc_stg = nc.dram_tensor("hsc_stg", [128, KCH], f32).ap()
        g_hsc = nc.dram_tensor("g_hsc", [NCORES * 128, KCH], f32,
                               addr_space="Shared").ap()

    # per-token-chunk attention output + allgather buffers (separate tensors so
    # each collective only depends on its own chunk's writes)
    ao_ch = [nc.dram_tensor(f"ao{i}", [DQ, CH_W], bf16).ap()
             for i in range(NCHUNK)]
    g_ch = [nc.dram_tensor(f"g{i}", [NCORES * DQ, CH_W], bf16,
                           addr_space="Shared").ap()
            for i in range(NCHUNK)]

    wq_3d = wq.rearrange("(ko ki) d -> ki ko d", ki=128)
    wk_3d = wk.rearrange("(ko ki) d -> ki ko d", ki=128)
    wv_3d = wv.rearrange("(ko ki) d -> ki ko d", ki=128)
    wo_3d = wo.rearrange("(ko ki) d -> ki ko d", ki=128)
    # g_hs rows = c*H + ko*128 + ki -> chunk index c*KCH + ko
    g_hs_3d = g_hs.rearrange("(ko ki) t -> ki ko t", ki=128)
    g_3d = [g.rearrange("(ko ki) t -> ki ko t", ki=128) for g in g_ch]

    # causal-packed pT row offsets: row kt covers qt in [kt*128, S)
    offs = []
    o = 0
    for kt in range(NB_S):
        offs.append(o)
        o += S - kt * 128
    PT_COLS = o  # 17408

    from contextlib import ExitStack
    with tile.TileContext(nc) as tc, ExitStack() as ctx:
        consts = ctx.enter_context(tc.tile_pool(name="consts", bufs=1))
        wpool = ctx.enter_context(tc.tile_pool(name="wpool", bufs=6))
        hs_pool = ctx.enter_context(tc.tile_pool(name="hs", bufs=8))
        if HS_INT8:
            hsq_pool = ctx.enter_context(tc.tile_pool(name="hsq", bufs=8))
        qkv_pool = ctx.enter_context(tc.tile_pool(name="qkv", bufs=1))
        pt_pool = ctx.enter_context(tc.tile_pool(name="pt", bufs=1))
        rope_pool = ctx.enter_context(tc.tile_pool(name="rope", bufs=2))
        ao_pool = ctx.enter_context(tc.tile_pool(name="ao", bufs=2))
        aorow_pool = ctx.enter_context(tc.tile_pool(name="aorow", bufs=1))
        g_pool = ctx.enter_context(tc.tile_pool(name="gp", bufs=3))
        wo_pool = ctx.enter_context(tc.tile_pool(name="wop", bufs=2))
        out_pool = ctx.enter_context(tc.tile_pool(name="outp", bufs=2))
        ps = ctx.enter_context(tc.tile_pool(name="ps", bufs=8, space="PSUM"))

        # reconstruct full activations/rope tables from the per-core shards.
        # small gathers first (unblock RoPE tables / dequant scales), then hs.
        groups = [list(range(NCORES))]
        nc.sync.dma_start(out=cs_stg[:, :], in_=cs_sh[:, :])
        if HS_INT8:
            nc.sync.dma_start(out=hsc_stg[:, :], in_=hsc_sh[:, :])
        nc.sync.dma_start(out=hs_stg[:, :], in_=hs_sh[:, :])
        nc.gpsimd.collective_compute(
            "AllGather", mybir.AluOpType.bypass, replica_groups=groups,
            ins=[cs_stg[:, :].opt()], outs=[g_cs[:, :].opt()])
        if HS_INT8:
            nc.gpsimd.collective_compute(
                "AllGather", mybir.AluOpType.bypass, replica_groups=groups,
                ins=[hsc_stg[:, :].opt()], outs=[g_hsc[:, :].opt()])
        nc.gpsimd.collective_compute(
            "AllGather", mybir.AluOpType.bypass, replica_groups=groups,
            ins=[hs_stg[:, :].opt()], outs=[g_hs[:, :].opt()])

        # qkv weights (first two d-blocks gate the first matmul group)
        def _load_w(m):
            wt = wpool.tile([128, KCH, 128], bf16, tag="w", name=f"w{m}")
            if m < HQ:
                nc.sync.dma_start(out=wt[:], in_=wq_3d[:, :, m * 128:(m + 1) * 128])
            elif m == HQ:
                nc.sync.dma_start(out=wt[:], in_=wk_3d[:, :, :])
            else:
                nc.sync.dma_start(out=wt[:], in_=wv_3d[:, :, :])
            return wt

        w_sb = [_load_w(0), _load_w(1)]

        # constants: identity (for PE transpose) + upper-tri causal keep-mask
        cst = consts.tile([128, 256], bf16, tag="cst")
        ident = cst[:, 0:128]
        tri = cst[:, 128:256]
        make_identity(nc, ident)
        make_upper_triangular(nc, tri, val=1.0, diag=True)

        for b in range(B):
            qT = qkv_pool.tile([128, HQ, S], bf16, tag="qT")
            kT = qkv_pool.tile([128, S], bf16, tag="kT")
            v_sb = qkv_pool.tile([128, NB_S, HD + 1], bf16, tag="v")
            nc.vector.memset(v_sb[:, :, HD:HD + 1], 1.0)
            # per-batch cos/sin table slices from the gathered shards
            cos_sb = qkv_pool.tile([128, S], bf16, tag="cos")
            sin_sb = qkv_pool.tile([128, S], bf16, tag="sin")
            for i in range(NSC):
                gc = b * NSC + i
                nc.gpsimd.dma_start(
                    out=cos_sb[:, i * TSH:(i + 1) * TSH],
                    in_=g_cs[gc * 2 * HD:gc * 2 * HD + HD, :])
                nc.gpsimd.dma_start(
                    out=sin_sb[:, i * TSH:(i + 1) * TSH],
                    in_=g_cs[gc * 2 * HD + HD:(gc + 1) * 2 * HD, :])
            if HS_INT8:
                # per-row dequant scales for this batch's 4 token chunks
                hsc_sb = qkv_pool.tile([128, NSC, KCH], f32, tag="hsc")
                for i in range(NSC):
                    gc = b * NSC + i
                    nc.gpsimd.dma_start(
                        out=hsc_sb[:, i, :],
                        in_=g_hsc[gc * 128:(gc + 1) * 128, :])

            # ---- projections: qT/kT/vT for this batch ----
            KO4 = 4
            for scid in range(NSC):
                gc = b * NSC + scid
                base = gc * KCH
                tloc = scid * TSH
                hs_t = []
                for oc in range(KCH // KO4):
                    if HS_INT8:
                        hq = hsq_pool.tile([128, KO4, TSH], i8, tag="hsq")
                        nc.sync.dma_start(
                            out=hq[:],
                            in_=g_hs_3d[:, base + oc * KO4:
                                        base + (oc + 1) * KO4, :])
                        ht = hs_pool.tile([128, KO4, TSH], bf16, tag="hs")
                        for j in range(KO4):
                            kk = oc * KO4 + j
                            nc.vector.tensor_scalar_mul(
                                ht[:, j, :], hq[:, j, :],
                                hsc_sb[:, scid, kk:kk + 1])
                    else:
                        ht = hs_pool.tile([128, KO4, TSH], bf16, tag="hs")
                        nc.sync.dma_start(
                            out=ht[:],
                            in_=g_hs_3d[:, base + oc * KO4:
                                        base + (oc + 1) * KO4, :])
                    hs_t.append(ht)
                if b == 0 and scid == 0:
                    for m in range(2, 6):
                        w_sb.append(_load_w(m))

                # 6 output d-blocks: q0..q3, k, v
                for grp in range(6):
                    p = ps.tile([128, TSH], f32, tag="ps", name="pj")
                    for k in range(KCH):
                        nc.tensor.matmul(
                            p[:], w_sb[grp][:, k, :],
                            hs_t[k // KO4][:, k % KO4, :],
                            start=(k == 0), stop=(k == KCH - 1))
                    m = grp
                    if m < 5:  # q heads 0..3 and k: RoPE
                        raw = rope_pool.tile([128, TSH], bf16, tag="raw")
                        nc.vector.tensor_copy(raw[:], p[:])
                        swp = rope_pool.tile([128, TSH], bf16, tag="swp", bufs=1)
                        nc.gpsimd.dma_start(out=swp[0:64, :],
                                            in_=raw[64:128, :])
                        nc.gpsimd.dma_start(out=swp[64:128, :],
                                            in_=raw[0:64, :])
                        ta = rope_pool.tile([128, TSH], bf16, tag="ta", bufs=1)
                        nc.vector.tensor_mul(ta[:], p[:],
                                             cos_sb[:, tloc:tloc + TSH])
                        nc.vector.tensor_mul(swp[:], swp[:],
                                             sin_sb[:, tloc:tloc + TSH])
                        dst = (qT[:, m, tloc:tloc + TSH] if m < HQ
                               else kT[:, tloc:tloc + TSH])
                        nc.vector.tensor_add(dst, ta[:], swp[:])
                    else:  # v: copy then transpose into [t, d] layout
                        vt_tmp = rope_pool.tile([128, TSH], bf16, tag="raw")
                        nc.vector.tensor_copy(vt_tmp[:], p[:])
                        for i2 in range(TSH // 128):
                            ktb = tloc // 128 + i2
                            tp = ps.tile([128, 128], bf16, tag="ps")
                            nc.tensor.transpose(
                                tp[:],
                                vt_tmp[:, i2 * 128:(i2 + 1) * 128],
                                ident)
                            nc.vector.tensor_copy(v_sb[:, ktb, 0:HD],
                                                  tp[:])

            # ---- attention per head ----
            for h in range(HQ):
                pT = pt_pool.tile([128, PT_COLS], bf16, tag="pT")
                # scoresT rows (kt on partitions), exp into pT
                for kt in range(NB_S):
                    qs = kt * 128
                    while qs < S:
                        w = min(512, S - qs)
                        sp = ps.tile([128, TC_W], f32, tag="ps", name="sp")
                        nc.tensor.matmul(sp[:, :w],
                                         kT[:, kt * 128:(kt + 1) * 128],
                                         qT[:, h, qs:qs + w],
                                         start=True, stop=True)
                        nc.scalar.activation(
                            out=pT[:, offs[kt] + qs - kt * 128:
                                   offs[kt] + qs - kt * 128 + w],
                            in_=sp[:, :w],
                            func=mybir.ActivationFunctionType.Exp,
                            scale=SM_SCALE)
                        qs += w
                    # mask the diagonal block (keep kt<=qt)
                    nc.vector.tensor_mul(pT[:, offs[kt]:offs[kt] + 128],
                                         pT[:, offs[kt]:offs[kt] + 128], tri)

                # PV with deferred normalization (col HD = row sums l)
                ao_row = aorow_pool.tile([128, S], bf16, tag="aorow")
                for qtb in range(NB_S):
                    pv = ps.tile([128, TC_W], f32, tag="ps", name="pv")
                    for kt in range(qtb + 1):
                        lhsT = pT[:, offs[kt] + (qtb - kt) * 128:
                                  offs[kt] + (qtb - kt) * 128 + 128]
                        nc.tensor.matmul(pv[:, :HD + 1], lhsT, v_sb[:, kt, :],
                                         start=(kt == 0), stop=(kt == qtb))
                    rl = ao_pool.tile([128, 1], f32, tag="rl")
                    nc.vector.reciprocal(rl[:], pv[:, HD:HD + 1])
                    ao = ao_pool.tile([128, HD], bf16, tag="aob", bufs=1)
                    nc.vector.tensor_scalar_mul(ao[:], pv[:, 0:HD], rl[:])
                    tp = ps.tile([128, 128], bf16, tag="ps", name="tp")
                    nc.tensor.transpose(tp[:], ao[:], ident)
                    nc.vector.tensor_copy(
                        ao_row[:, qtb * 128:(qtb + 1) * 128], tp[:])
                # store this head's transposed output, split by gather chunk
                for half in range(S // CH_W):
                    ci = (b * S + half * CH_W) // CH_W
                    nc.scalar.dma_start(
                        out=ao_ch[ci][h * 128:(h + 1) * 128, :],
                        in_=ao_row[:, half * CH_W:(half + 1) * CH_W])

            # allgather this batch's chunks as soon as attention produced them
            for half in range(S // CH_W):
                ci = (b * S + half * CH_W) // CH_W
                nc.gpsimd.collective_compute(
                    "AllGather", mybir.AluOpType.bypass,
                    replica_groups=[list(range(NCORES))],
                    ins=[ao_ch[ci][:, :].opt()],
                    outs=[g_ch[ci][:, :].opt()])

        # ---- o_proj: outT[f, t] += Wo_c[d, f].T @ gathered[d, t] ----
        DP = 4   # d-chunks per gathered DMA batch (sync queue)
        DPW = 4  # d-chunks per wo DMA batch (gpsimd queue)
        # per-head-block scale accumulators [128, NOG], DMA'd once at end
        s_sb = [out_pool.tile([128, NOG], f32, tag="ssb", name=f"s{f}",
                              bufs=HQ)
                for f in range(HQ)]
        GPT = TC_W // OG  # scale groups per token chunk
        for tcid in range(T // TC_W):
            ci = tcid * TC_W // CH_W
            toff = (tcid * TC_W) % CH_W
            psums = []
            for f in range(HQ):
                p = ps.tile([128, TC_W], f32, tag="ps")
                psums.append(p)
            wo_ts = []
            for wp in range(KCH // DPW):
                wo_t = wo_pool.tile([128, DPW, DQ], bf16, tag="wo")
                nc.gpsimd.dma_start(
                    out=wo_t[:], in_=wo_3d[:, wp * DPW:(wp + 1) * DPW, :])
                wo_ts.append(wo_t)
            for dp in range(KCH // DP):
                g_t = g_pool.tile([128, DP, TC_W], bf16, tag="g")
                nc.sync.dma_start(
                    out=g_t[:],
                    in_=g_3d[ci][:, dp * DP:(dp + 1) * DP, toff:toff + TC_W])
                for dd in range(DP):
                    d = dp * DP + dd
                    for f in range(HQ):
                        nc.tensor.matmul(
                            psums[f][:],
                            wo_ts[d // DPW][:, d % DPW, f * 128:(f + 1) * 128],
                            g_t[:, dd, :],
                            start=(dp == 0 and dd == 0),
                            stop=(dp == KCH // DP - 1 and dd == DP - 1))
            for f in range(HQ):
                amax = out_pool.tile([128, GPT], f32, tag="amax")
                for j in range(GPT):
                    nc.vector.tensor_reduce(
                        amax[:, j:j + 1], psums[f][:, j * OG:(j + 1) * OG],
                        axis=mybir.AxisListType.X,
                        op=mybir.AluOpType.max, apply_absolute_value=True)
                nc.vector.tensor_copy(
                    s_sb[f][:, tcid * GPT:(tcid + 1) * GPT], amax[:])
                rcp = out_pool.tile([128, GPT], f32, tag="rcp")
                nc.vector.reciprocal(rcp[:], amax[:])
                q127 = out_pool.tile([128, GPT], f32, tag="q127")
                nc.scalar.activation(
                    out=q127[:], in_=rcp[:],
                    func=mybir.ActivationFunctionType.Copy, scale=127.0)
                o_i8 = out_pool.tile([128, TC_W], i8, tag="oi8")
                for j in range(GPT):
                    nc.vector.tensor_scalar_mul(
                        o_i8[:, j * OG:(j + 1) * OG],
                        psums[f][:, j * OG:(j + 1) * OG], q127[:, j:j + 1])
                nc.scalar.dma_start(
                    out=outT_i8[f * 128:(f + 1) * 128,
                                tcid * TC_W:(tcid + 1) * TC_W],
                    in_=o_i8[:])
        for f in range(HQ):
            nc.scalar.dma_start(
                out=oscale[f * 128:(f + 1) * 128, :], in_=s_sb[f][:])

    nc.compile()
    return nc


def _get_nc():
    if "nc" not in _state:
        _state["nc"] = _build()
    return _state["nc"]


def _shard0(full, width):
    """[R, T] -> [NCORES*R, width] stacking per-core token slices on dim 0."""
    R = full.shape[0]
    return np.ascontiguousarray(
        full.reshape(R, NCORES, width).transpose(1, 0, 2)).reshape(
            NCORES * R, width)


def _pool():
    if "pool" not in _state:
        from concurrent.futures import ThreadPoolExecutor
        _state["pool"] = ThreadPoolExecutor(NCORES)
    return _state["pool"]


def _prep_hs(hidden_states):
    """[B,S,H] fp32 -> per-core token shards.

    HS_INT8: int8 values + per-(row, shard) scale = absmax/127; else bf16."""
    a = np.asarray(hidden_states, dtype=np.float32).reshape(NCORES, TSH, H)
    if not HS_INT8:
        # fp32->bf16 RNE via integer ops (they release the GIL, unlike the
        # ml_dtypes astype, so the 8 shards convert in parallel threads)
        out = np.empty((NCORES, H, TSH), np.uint16)

        def one(c):
            v = np.ascontiguousarray(a[c].T).view(np.uint32)
            np.right_shift(v + 0x7FFF + ((v >> 16) & 1), 16, out=v)
            out[c] = v.astype(np.uint16)

        list(_pool().map(one, range(NCORES)))
        return {"hs_sh": out.reshape(NCORES * H, TSH).view(ml_dtypes.bfloat16)}
    q = np.empty((NCORES, H, TSH), np.int8)
    s = np.empty((NCORES, 128, KCH), np.float32)

    def one(c):
        x = np.ascontiguousarray(a[c].T)  # [H, TSH]
        am = np.abs(x).max(axis=1)
        np.maximum(am, 1e-30, out=am)
        np.rint(x * (127.0 / am)[:, None], out=x)
        q[c] = x  # exact integers, cast is lossless
        # device wants scales as [ki, ko]; h = ko*128 + ki
        s[c] = (am * (1.0 / 127.0)).reshape(KCH, 128).T

    list(_pool().map(one, range(NCORES)))
    return {"hs_sh": q.reshape(NCORES * H, TSH),
            "hsc_sh": s.reshape(NCORES * 128, KCH)}


def _prep_consts(Wq, Wk, Wv, Wo, position_ids):
    bf16 = ml_dtypes.bfloat16
    inv = (1.0 / (ROPE_THETA ** (np.arange(0, HD, 2, dtype=np.float32) / HD)))
    pos = np.asarray(position_ids).reshape(T).astype(np.float32)
    fr = pos[None, :] * inv[:, None]  # [64, T]
    cos = np.cos(fr)
    sin = np.sin(fr)
    cs = np.concatenate([cos, cos, -sin, sin], axis=0).astype(bf16)  # [256, T]

    def wcat(Wfull, wd):
        Wfull = np.asarray(Wfull, dtype=np.float32)
        R = Wfull.shape[0]
        return np.ascontiguousarray(
            Wfull.reshape(R, NCORES, wd).transpose(1, 0, 2)).astype(
                bf16).reshape(NCORES * R, wd)

    return {
        "wq": wcat(Wq, DQ),
        "wk": wcat(Wk, HD),
        "wv": wcat(Wv, HD),
        "wo": wcat(Wo, DQ),
        "cs_sh": _shard0(cs, TSH),
    }


def _prep_inputs(hidden_states, Wq, Wk, Wv, Wo, position_ids):
    out = {"hs_sh": _prep_hs(hidden_states)}
    out.update(_prep_consts(Wq, Wk, Wv, Wo, position_ids))
    return out


def _get_runner():
    """Build the sharded jit once; reuse across kernel() calls."""
    if "runner" in _state:
        return _state["runner"]

    import jax
    import jax.numpy as jnp
    import concourse.mybir as mybir
    from concourse import bass2jax
    from jax.sharding import Mesh, PartitionSpec, NamedSharding
    from jax.experimental.shard_map import shard_map

    nc = _get_nc()
    bass2jax.install_neuronx_cc_hook()

    in_names = []
    out_names = []
    out_avals = []
    zero_shapes = []
    for alloc in nc.m.functions[0].allocations:
        if not isinstance(alloc, mybir.MemoryLocationSet):
            continue
        name = alloc.memorylocations[0].name
        if alloc.kind == "ExternalInput":
            if nc.partition_id_tensor is None or name != nc.partition_id_tensor.name:
                in_names.append(name)
        elif alloc.kind == "ExternalOutput":
            shape = tuple(alloc.tensor_shape)
            dtype = mybir.dt.np(alloc.dtype)
            out_names.append(name)
            out_avals.append(jax.core.ShapedArray(shape, dtype))
            zero_shapes.append(((NCORES * shape[0],) + shape[1:], dtype))

    n_params = len(in_names)
    n_outs = len(out_avals)
    all_in_names = list(in_names) + list(out_names)
    if nc.partition_id_tensor is not None:
        all_in_names.append(nc.partition_id_tensor.name)

    def _body(*args):
        operands = list(args)
        if nc.partition_id_tensor is not None:
            operands.append(bass2jax.partition_id_tensor())
        outs = bass2jax._bass_exec_p.bind(
            *operands,
            out_avals=tuple(out_avals),
            in_names=tuple(all_in_names),
            out_names=tuple(out_names),
            lowering_input_output_aliases=(),
            sim_require_finite=True,
            sim_require_nnan=True,
            nc=nc,
        )
        return tuple(outs)

    devices = jax.devices()[:NCORES]
    mesh = Mesh(np.asarray(devices), ("core",))
    in_specs = (PartitionSpec("core"),) * (n_params + n_outs)
    out_specs = (PartitionSpec("core"),) * n_outs
    donate = tuple(range(n_params, n_params + n_outs))
    sharded = jax.jit(
        shard_map(_body, mesh=mesh, in_specs=in_specs, out_specs=out_specs,
                  check_rep=False),
        donate_argnums=donate, keep_unused=True)

    sh = NamedSharding(mesh, PartitionSpec("core"))
    zeros_fn = jax.jit(
        lambda: tuple(jnp.zeros(s, d) for s, d in zero_shapes),
        out_shardings=tuple(sh for _ in zero_shapes))

    import os
    import time
    dbg = bool(os.environ.get("KERN_TIMING"))

    def run(cat_map):
        # values may be host numpy arrays (transferred now) or cached
        # device-resident jax arrays (no transfer)
        t0 = time.perf_counter()
        ins = [cat_map[name] for name in in_names]
        # donated output buffers: recycle the previous call's (fully
        # overwritten) device outputs; fall back to fresh device zeros
        zs = _state.pop("recycle", None)
        if zs is None:
            zs = zeros_fn()
        if dbg:
            th0 = time.perf_counter()
            import jax as _jax
            ins[0] = _jax.device_put(ins[0], sh)
            ins[0].block_until_ready()
            t1 = time.perf_counter()
            print(f"    [run] donbuf {th0-t0:.3f}s  hs H2D {t1-th0:.3f}s",
                  flush=True)
        out_arrs = sharded(*ins, *zs)
        _state["recycle"] = tuple(out_arrs)
        return {name: out_arrs[i] for i, name in enumerate(out_names)}

    _state["sharding"] = sh
    _state["runner"] = run
    return run


def kernel(hidden_states, Wq, Wk, Wv, Wo, attention_mask, position_ids):
    """Weights + RoPE tables are kept device-resident across calls, guarded by
    a full content comparison against stashed host copies (so a call with new
    weights re-uploads). Activations are prepped + transferred every call."""
    import jax
    import os
    import time

    dbg = bool(os.environ.get("KERN_TIMING"))
    t0 = time.perf_counter()
    run = _get_runner()
    key_arrays = [np.asarray(x) for x in (Wq, Wk, Wv, Wo, position_ids)]
    wc = _state.get("wcache")
    # fast path: same array objects as last call; else full content compare
    hit = wc is not None and (
        all(a is b for a, b in zip(key_arrays, wc["orig"]))
        or all(a.shape == b.shape and a.dtype == b.dtype and np.array_equal(a, b)
               for a, b in zip(key_arrays, wc["host"])))
    if dbg:
        t1 = time.perf_counter()
        print(f"    [kern] wcheck {t1-t0:.3f}s hit={hit}", flush=True)
    if not hit:
        consts = _prep_consts(Wq, Wk, Wv, Wo, position_ids)
        dev = {k: jax.device_put(v, _state["sharding"])
               for k, v in consts.items()}
        for v in dev.values():
            v.block_until_ready()
        wc = {"orig": key_arrays, "host": [a.copy() for a in key_arrays],
              "dev": dev}
        _state["wcache"] = wc

    t2 = time.perf_counter()
    cat = _prep_hs(hidden_states)
    cat.update(wc["dev"])
    if dbg:
        t3 = time.perf_counter()
        print(f"    [kern] hsprep {t3-t2:.3f}s", flush=True)
    devarrs = run(cat)
    t4 = time.perf_counter()
    # fetch every device shard concurrently (overlaps the per-array sync
    # round-trips and the dequant work with the D2H stream), dequantize and
    # transpose-assemble: out[t, fg] = i8[fg, t] * sc[fg, t//OG]; core c owns
    # output feature columns [c*DQ, (c+1)*DQ)
    out = np.empty((T, H), dtype=np.float32)

    def by_core(garr):
        m = {}
        for s in garr.addressable_shards:
            m[(s.index[0].start or 0) // DQ] = s.data
        return m

    i8_by_c = by_core(devarrs["outT_i8"])
    sc_by_c = by_core(devarrs["oscale"])

    def onec(c):
        scc = np.asarray(sc_by_c[c]).astype(np.float32) * (1.0 / 127.0)
        i8c = np.asarray(i8_by_c[c])  # [DQ, T] int8
        col = c * DQ
        for j in range(NOG):
            np.multiply(i8c[:, j * OG:(j + 1) * OG].T.astype(np.float32),
                        scc[:, j][None, :],
                        out=out[j * OG:(j + 1) * OG, col:col + DQ])

    list(_pool().map(onec, range(NCORES)))
    out = out.reshape(B, S, H)
    if dbg:
        t5 = time.perf_counter()
        print(f"    [kern] assemble {t5-t4:.3f}s", flush=True)
    return out


# revision 44
# speedup vs baseline: 1.2690x; 1.2690x over previous
"""Llama attention layer (B=2, S=2048, H=4096, 32 q heads / 8 kv heads, HD=128)
on 8 Trainium2 NeuronCores, tensor-parallel over heads.

Host->device traffic is the wall-clock bottleneck (axon-tunneled cores), so
inputs are sharded to minimize bytes over the tunnel:
  - hidden_states: each core receives a distinct 512-token slice of hsT
    (4MB/core instead of a full 32MB replica); an on-device AllGather
    reconstructs the full [H, T] activation in shared DRAM before the
    projections.
  - cos/sin RoPE tables: same 512-token sharding + AllGather (0.25MB/core).
  - weights: per-core head shards (Wq/Wo 512 cols, Wk/Wv 128 cols), bf16.
  - donated output buffers are created device-side (jnp.zeros under jit),
    not transferred.

Per core c (SPMD, identical program, different data):
  - weights: Wq[:, 512c:512c+512], Wk/Wv[:, 128c:128c+128], Wo[:, 512c:512c+512]
  - qT/kT/vT projections (weights stationary, transposed outputs), RoPE on DVE
    with host-precomputed cos/sin tables
  - attention computed transposed (scoresT = [k-tokens, q-tokens]) so exp'd
    score tiles feed the PV matmul as lhsT with no transposes; softmax
    normalization deferred via a ones-column appended to V (row sums land
    per-partition); causal = only lower blocks + masked diagonal tile
  - AllGather of per-core attention outputs in 4 token chunks (overlapped with
    compute), then column-sharded o_proj producing the transposed output slice
All matmuls bf16 with fp32 PSUM accumulation.
"""

import sys

sys.path.insert(0, "/opt/trn_rl_repo")

import numpy as np
import ml_dtypes

B, S, H = 2, 2048, 4096
NQ, NKV, HD = 32, 8, 128
T = B * S  # 4096 global tokens, j = b*S + s
NCORES = 8
HQ = NQ // NCORES  # 4 q heads per core
DQ = HQ * HD  # 512 q dims per core
ROPE_THETA = 10000.0
SM_SCALE = 1.0 / float(np.sqrt(HD))

NB_S = S // 128  # 16 token blocks per batch
KCH = H // 128  # 32 contraction chunks
TC_W = 512  # token chunk width in attention/o_proj phases
TSH = T // NCORES  # 512-token hs shard per core
NSC = S // TSH  # 4 hs shards per batch
NCHUNK = 4  # allgather token chunks (attention output)
CH_W = T // NCHUNK  # 1024 tokens per gather chunk
OG = 128  # output int8 scale group width (tokens)
NOG = T // OG  # scale groups per output row
# int8 hidden_states transfer was tried and reverted: quantization noise on
# q/k scores is amplified by softmax (measured rel err 1.96e-2 vs the 2e-2
# gate) for only ~0.12s net saving. Keep activations bf16 on the wire.
HS_INT8 = False

_state = {}


def _build():
    import concourse.bass as bass
    import concourse.mybir as mybir
    import concourse.tile as tile
    from concourse import bacc
    from concourse.masks import make_identity, make_upper_triangular

    f32 = mybir.dt.float32
    bf16 = mybir.dt.bfloat16

    nc = bacc.Bacc("TRN2", target_bir_lowering=False, debug=False,
                   num_devices=NCORES)

    i8 = mybir.dt.int8
    hs_dt = i8 if HS_INT8 else bf16
    hs_sh = nc.dram_tensor("hs_sh", [H, TSH], hs_dt, kind="ExternalInput").ap()
    if HS_INT8:
        # per-row dequant scales, host-transposed to [ki, ko] so SBUF loads
        # are contiguous per partition
        hsc_sh = nc.dram_tensor("hsc_sh", [128, KCH], f32,
                                kind="ExternalInput").ap()
    wq = nc.dram_tensor("wq", [H, DQ], bf16, kind="ExternalInput").ap()
    wk = nc.dram_tensor("wk", [H, HD], bf16, kind="ExternalInput").ap()
    wv = nc.dram_tensor("wv", [H, HD], bf16, kind="ExternalInput").ap()
    wo = nc.dram_tensor("wo", [H, DQ], bf16, kind="ExternalInput").ap()
    cs_sh = nc.dram_tensor("cs_sh", [2 * HD, TSH], bf16,
                           kind="ExternalInput").ap()
    # output wire format: int8 with per-(row, OG-token-group) absmax scales
    # (halves D2H bytes; host dequantizes)
    outT_i8 = nc.dram_tensor("outT_i8", [DQ, T], i8, kind="ExternalOutput").ap()
    oscale = nc.dram_tensor("oscale", [DQ, NOG], f32,
                            kind="ExternalOutput").ap()

    # gathered activations / rope tables (written once by the start-of-kernel
    # AllGathers, read by projections). Collectives cannot read IO tensors, so
    # the input shards are staged through internal DRAM first.
    hs_stg = nc.dram_tensor("hs_stg", [H, TSH], hs_dt).ap()
    cs_stg = nc.dram_tensor("cs_stg", [2 * HD, TSH], bf16).ap()
    g_hs = nc.dram_tensor("g_hs", [NCORES * H, TSH], hs_dt,
                          addr_space="Shared").ap()
    g_cs = nc.dram_tensor("g_cs", [NCORES * 2 * HD, TSH], bf16,
                          addr_space="Shared").ap()
    if HS_INT8:
        hsc_stg = nc.dram_tensor("hsc_stg", [128, KCH], f32).ap()
        g_hsc = nc.dram_tensor("g_hsc", [NCORES * 128, KCH], f32,
                               addr_space="Shared").ap()

    # per-token-chunk attention output + allgather buffers (separate tensors so
    # each collective only depends on its own chunk's writes)
    ao_ch = [nc.dram_tensor(f"ao{i}", [DQ, CH_W], bf16).ap()
             for i in range(NCHUNK)]
    g_ch = [nc.dram_tensor(f"g{i}", [NCORES * DQ, CH_W], bf16,
                           addr_space="Shared").ap()
            for i in range(NCHUNK)]

    wq_3d = wq.rearrange("(ko ki) d -> ki ko d", ki=128)
    wk_3d = wk.rearrange("(ko ki) d -> ki ko d", ki=128)
    wv_3d = wv.rearrange("(ko ki) d -> ki ko d", ki=128)
    wo_3d = wo.rearrange("(ko ki) d -> ki ko d", ki=128)
    # g_hs rows = c*H + ko*128 + ki -> chunk index c*KCH + ko
    g_hs_3d = g_hs.rearrange("(ko ki) t -> ki ko t", ki=128)
    g_3d = [g.rearrange("(ko ki) t -> ki ko t", ki=128) for g in g_ch]

    # causal-packed pT row offsets: row kt covers qt in [kt*128, S)
    offs = []
    o = 0
    for kt in range(NB_S):
        offs.append(o)
        o += S - kt * 128
    PT_COLS = o  # 17408

    from contextlib import ExitStack
    with tile.TileContext(nc) as tc, ExitStack() as ctx:
        consts = ctx.enter_context(tc.tile_pool(name="consts", bufs=1))
        wpool = ctx.enter_context(tc.tile_pool(name="wpool", bufs=6))
        hs_pool = ctx.enter_context(tc.tile_pool(name="hs", bufs=8))
        if HS_INT8:
            hsq_pool = ctx.enter_context(tc.tile_pool(name="hsq", bufs=8))
        qkv_pool = ctx.enter_context(tc.tile_pool(name="qkv", bufs=1))
        pt_pool = ctx.enter_context(tc.tile_pool(name="pt", bufs=1))
        rope_pool = ctx.enter_context(tc.tile_pool(name="rope", bufs=2))
        ao_pool = ctx.enter_context(tc.tile_pool(name="ao", bufs=2))
        aorow_pool = ctx.enter_context(tc.tile_pool(name="aorow", bufs=1))
        g_pool = ctx.enter_context(tc.tile_pool(name="gp", bufs=3))
        wo_pool = ctx.enter_context(tc.tile_pool(name="wop", bufs=2))
        out_pool = ctx.enter_context(tc.tile_pool(name="outp", bufs=2))
        ps = ctx.enter_context(tc.tile_pool(name="ps", bufs=8, space="PSUM"))

        # reconstruct full activations/rope tables from the per-core shards.
        # small gathers first (unblock RoPE tables / dequant scales), then hs.
        groups = [list(range(NCORES))]
        nc.sync.dma_start(out=cs_stg[:, :], in_=cs_sh[:, :])
        if HS_INT8:
            nc.sync.dma_start(out=hsc_stg[:, :], in_=hsc_sh[:, :])
        nc.sync.dma_start(out=hs_stg[:, :], in_=hs_sh[:, :])
        nc.gpsimd.collective_compute(
            "AllGather", mybir.AluOpType.bypass, replica_groups=groups,
            ins=[cs_stg[:, :].opt()], outs=[g_cs[:, :].opt()])
        if HS_INT8:
            nc.gpsimd.collective_compute(
                "AllGather", mybir.AluOpType.bypass, replica_groups=groups,
                ins=[hsc_stg[:, :].opt()], outs=[g_hsc[:, :].opt()])
        nc.gpsimd.collective_compute(
            "AllGather", mybir.AluOpType.bypass, replica_groups=groups,
            ins=[hs_stg[:, :].opt()], outs=[g_hs[:, :].opt()])

        # qkv weights (first two d-blocks gate the first matmul group)
        def _load_w(m):
            wt = wpool.tile([128, KCH, 128], bf16, tag="w", name=f"w{m}")
            if m < HQ:
                nc.sync.dma_start(out=wt[:], in_=wq_3d[:, :, m * 128:(m + 1) * 128])
            elif m == HQ:
                nc.sync.dma_start(out=wt[:], in_=wk_3d[:, :, :])
            else:
                nc.sync.dma_start(out=wt[:], in_=wv_3d[:, :, :])
            return wt

        w_sb = [_load_w(0), _load_w(1)]

        # constants: identity (for PE transpose) + upper-tri causal keep-mask
        cst = consts.tile([128, 256], bf16, tag="cst")
        ident = cst[:, 0:128]
        tri = cst[:, 128:256]
        make_identity(nc, ident)
        make_upper_triangular(nc, tri, val=1.0, diag=True)

        for b in range(B):
            qT = qkv_pool.tile([128, HQ, S], bf16, tag="qT")
            kT = qkv_pool.tile([128, S], bf16, tag="kT")
            v_sb = qkv_pool.tile([128, NB_S, HD + 1], bf16, tag="v")
            nc.vector.memset(v_sb[:, :, HD:HD + 1], 1.0)
            # per-batch cos/sin table slices from the gathered shards
            cos_sb = qkv_pool.tile([128, S], bf16, tag="cos")
            sin_sb = qkv_pool.tile([128, S], bf16, tag="sin")
            for i in range(NSC):
                gc = b * NSC + i
                nc.gpsimd.dma_start(
                    out=cos_sb[:, i * TSH:(i + 1) * TSH],
                    in_=g_cs[gc * 2 * HD:gc * 2 * HD + HD, :])
                nc.gpsimd.dma_start(
                    out=sin_sb[:, i * TSH:(i + 1) * TSH],
                    in_=g_cs[gc * 2 * HD + HD:(gc + 1) * 2 * HD, :])
            if HS_INT8:
                # per-row dequant scales for this batch's 4 token chunks
                hsc_sb = qkv_pool.tile([128, NSC, KCH], f32, tag="hsc")
                for i in range(NSC):
                    gc = b * NSC + i
                    nc.gpsimd.dma_start(
                        out=hsc_sb[:, i, :],
                        in_=g_hsc[gc * 128:(gc + 1) * 128, :])

            # ---- projections: qT/kT/vT for this batch ----
            KO4 = 4
            for scid in range(NSC):
                gc = b * NSC + scid
                base = gc * KCH
                tloc = scid * TSH
                hs_t = []
                for oc in range(KCH // KO4):
                    if HS_INT8:
                        hq = hsq_pool.tile([128, KO4, TSH], i8, tag="hsq")
                        nc.sync.dma_start(
                            out=hq[:],
                            in_=g_hs_3d[:, base + oc * KO4:
                                        base + (oc + 1) * KO4, :])
                        ht = hs_pool.tile([128, KO4, TSH], bf16, tag="hs")
                        for j in range(KO4):
                            kk = oc * KO4 + j
                            nc.vector.tensor_scalar_mul(
                                ht[:, j, :], hq[:, j, :],
                                hsc_sb[:, scid, kk:kk + 1])
                    else:
                        ht = hs_pool.tile([128, KO4, TSH], bf16, tag="hs")
                        nc.sync.dma_start(
                            out=ht[:],
                            in_=g_hs_3d[:, base + oc * KO4:
                                        base + (oc + 1) * KO4, :])
                    hs_t.append(ht)
                if b == 0 and scid == 0:
                    for m in range(2, 6):
                        w_sb.append(_load_w(m))

                # 6 output d-blocks: q0..q3, k, v
                for grp in range(6):
                    p = ps.tile([128, TSH], f32, tag="ps", name="pj")
                    for k in range(KCH):
                        nc.tensor.matmul(
                            p[:], w_sb[grp][:, k, :],
                            hs_t[k // KO4][:, k % KO4, :],
                            start=(k == 0), stop=(k == KCH - 1))
                    m = grp
                    if m < 5:  # q heads 0..3 and k: RoPE
                        raw = rope_pool.tile([128, TSH], bf16, tag="raw")
                        nc.vector.tensor_copy(raw[:], p[:])
                        swp = rope_pool.tile([128, TSH], bf16, tag="swp", bufs=1)
                        nc.gpsimd.dma_start(out=swp[0:64, :],
                                            in_=raw[64:128, :])
                        nc.gpsimd.dma_start(out=swp[64:128, :],
                                            in_=raw[0:64, :])
                        ta = rope_pool.tile([128, TSH], bf16, tag="ta", bufs=1)
                        nc.vector.tensor_mul(ta[:], p[:],
                                             cos_sb[:, tloc:tloc + TSH])
                        nc.vector.tensor_mul(swp[:], swp[:],
                                             sin_sb[:, tloc:tloc + TSH])
                        dst = (qT[:, m, tloc:tloc + TSH] if m < HQ
                               else kT[:, tloc:tloc + TSH])
                        nc.vector.tensor_add(dst, ta[:], swp[:])
                    else:  # v: copy then transpose into [t, d] layout
                        vt_tmp = rope_pool.tile([128, TSH], bf16, tag="raw")
                        nc.vector.tensor_copy(vt_tmp[:], p[:])
                        for i2 in range(TSH // 128):
                            ktb = tloc // 128 + i2
                            tp = ps.tile([128, 128], bf16, tag="ps")
                            nc.tensor.transpose(
                                tp[:],
                                vt_tmp[:, i2 * 128:(i2 + 1) * 128],
                                ident)
                            nc.vector.tensor_copy(v_sb[:, ktb, 0:HD],
                                                  tp[:])

            # ---- attention per head ----
            for h in range(HQ):
                pT = pt_pool.tile([128, PT_COLS], bf16, tag="pT")
                # scoresT rows (kt on partitions), exp into pT
                for kt in range(NB_S):
                    qs = kt * 128
                    while qs < S:
                        w = min(512, S - qs)
                        sp = ps.tile([128, TC_W], f32, tag="ps", name="sp")
                        nc.tensor.matmul(sp[:, :w],
                                         kT[:, kt * 128:(kt + 1) * 128],
                                         qT[:, h, qs:qs + w],
                                         start=True, stop=True)
                        nc.scalar.activation(
                            out=pT[:, offs[kt] + qs - kt * 128:
                                   offs[kt] + qs - kt * 128 + w],
                            in_=sp[:, :w],
                            func=mybir.ActivationFunctionType.Exp,
                            scale=SM_SCALE)
                        qs += w
                    # mask the diagonal block (keep kt<=qt)
                    nc.vector.tensor_mul(pT[:, offs[kt]:offs[kt] + 128],
                                         pT[:, offs[kt]:offs[kt] + 128], tri)

                # PV with deferred normalization (col HD = row sums l)
                ao_row = aorow_pool.tile([128, S], bf16, tag="aorow")
                for qtb in range(NB_S):
                    pv = ps.tile([128, TC_W], f32, tag="ps", name="pv")
                    for kt in range(qtb + 1):
                        lhsT = pT[:, offs[kt] + (qtb - kt) * 128:
                                  offs[kt] + (qtb - kt) * 128 + 128]
                        nc.tensor.matmul(pv[:, :HD + 1], lhsT, v_sb[:, kt, :],
                                         start=(kt == 0), stop=(kt == qtb))
                    rl = ao_pool.tile([128, 1], f32, tag="rl")
                    nc.vector.reciprocal(rl[:], pv[:, HD:HD + 1])
                    ao = ao_pool.tile([128, HD], bf16, tag="aob", bufs=1)
                    nc.vector.tensor_scalar_mul(ao[:], pv[:, 0:HD], rl[:])
                    tp = ps.tile([128, 128], bf16, tag="ps", name="tp")
                    nc.tensor.transpose(tp[:], ao[:], ident)
                    nc.vector.tensor_copy(
                        ao_row[:, qtb * 128:(qtb + 1) * 128], tp[:])
                # store this head's transposed output, split by gather chunk
                for half in range(S // CH_W):
                    ci = (b * S + half * CH_W) // CH_W
                    nc.scalar.dma_start(
                        out=ao_ch[ci][h * 128:(h + 1) * 128, :],
                        in_=ao_row[:, half * CH_W:(half + 1) * CH_W])

            # allgather this batch's chunks as soon as attention produced them
            for half in range(S // CH_W):
                ci = (b * S + half * CH_W) // CH_W
                nc.gpsimd.collective_compute(
                    "AllGather", mybir.AluOpType.bypass,
                    replica_groups=[list(range(NCORES))],
                    ins=[ao_ch[ci][:, :].opt()],
                    outs=[g_ch[ci][:, :].opt()])

        # ---- o_proj: outT[f, t] += Wo_c[d, f].T @ gathered[d, t] ----
        DP = 4   # d-chunks per gathered DMA batch (sync queue)
        DPW = 4  # d-chunks per wo DMA batch (gpsimd queue)
        # per-head-block scale accumulators [128, NOG], DMA'd once at end
        s_sb = [out_pool.tile([128, NOG], f32, tag="ssb", name=f"s{f}",
                              bufs=HQ)
                for f in range(HQ)]
        GPT = TC_W // OG  # scale groups per token chunk
        for tcid in range(T // TC_W):
            ci = tcid * TC_W // CH_W
            toff = (tcid * TC_W) % CH_W
            psums = []
            for f in range(HQ):
                p = ps.tile([128, TC_W], f32, tag="ps")
                psums.append(p)
            wo_ts = []
            for wp in range(KCH // DPW):
                wo_t = wo_pool.tile([128, DPW, DQ], bf16, tag="wo")
                nc.gpsimd.dma_start(
                    out=wo_t[:], in_=wo_3d[:, wp * DPW:(wp + 1) * DPW, :])
                wo_ts.append(wo_t)
            for dp in range(KCH // DP):
                g_t = g_pool.tile([128, DP, TC_W], bf16, tag="g")
                nc.sync.dma_start(
                    out=g_t[:],
                    in_=g_3d[ci][:, dp * DP:(dp + 1) * DP, toff:toff + TC_W])
                for dd in range(DP):
                    d = dp * DP + dd
                    for f in range(HQ):
                        nc.tensor.matmul(
                            psums[f][:],
                            wo_ts[d // DPW][:, d % DPW, f * 128:(f + 1) * 128],
                            g_t[:, dd, :],
                            start=(dp == 0 and dd == 0),
                            stop=(dp == KCH // DP - 1 and dd == DP - 1))
            for f in range(HQ):
                amax = out_pool.tile([128, GPT], f32, tag="amax")
                for j in range(GPT):
                    nc.vector.tensor_reduce(
                        amax[:, j:j + 1], psums[f][:, j * OG:(j + 1) * OG],
                        axis=mybir.AxisListType.X,
                        op=mybir.AluOpType.max, apply_absolute_value=True)
                nc.vector.tensor_copy(
                    s_sb[f][:, tcid * GPT:(tcid + 1) * GPT], amax[:])
                rcp = out_pool.tile([128, GPT], f32, tag="rcp")
                nc.vector.reciprocal(rcp[:], amax[:])
                q127 = out_pool.tile([128, GPT], f32, tag="q127")
                nc.scalar.activation(
                    out=q127[:], in_=rcp[:],
                    func=mybir.ActivationFunctionType.Copy, scale=127.0)
                o_i8 = out_pool.tile([128, TC_W], i8, tag="oi8")
                for j in range(GPT):
                    nc.vector.tensor_scalar_mul(
                        o_i8[:, j * OG:(j + 1) * OG],
                        psums[f][:, j * OG:(j + 1) * OG], q127[:, j:j + 1])
                nc.scalar.dma_start(
                    out=outT_i8[f * 128:(f + 1) * 128,
                                tcid * TC_W:(tcid + 1) * TC_W],
                    in_=o_i8[:])
        for f in range(HQ):
            nc.scalar.dma_start(
                out=oscale[f * 128:(f + 1) * 128, :], in_=s_sb[f][:])

    nc.compile()
    return nc


def _get_nc():
    if "nc" not in _state:
        _state["nc"] = _build()
    return _state["nc"]


def _shard0(full, width):
    """[R, T] -> [NCORES*R, width] stacking per-core token slices on dim 0."""
    R = full.shape[0]
    return np.ascontiguousarray(
        full.reshape(R, NCORES, width).transpose(1, 0, 2)).reshape(
            NCORES * R, width)


def _pool():
    if "pool" not in _state:
        from concurrent.futures import ThreadPoolExecutor
        _state["pool"] = ThreadPoolExecutor(NCORES)
    return _state["pool"]


def _prep_hs(hidden_states):
    """[B,S,H] fp32 -> per-core token shards.

    HS_INT8: int8 values + per-(row, shard) scale = absmax/127; else bf16."""
    a = np.asarray(hidden_states, dtype=np.float32).reshape(NCORES, TSH, H)
    if not HS_INT8:
        # fp32->bf16 RNE via integer ops (they release the GIL, unlike the
        # ml_dtypes astype, so the 8 shards convert in parallel threads).
        # Persistent workspaces: no per-call allocations (8MB mmap/zeroing
        # per temp otherwise dominates).
        ws = _state.get("prep_ws")
        if ws is None:
            ws = {"f32": np.empty((NCORES, H, TSH), np.float32),
                  "tmp": np.empty((NCORES, H, TSH), np.uint32),
                  "u16": np.empty((NCORES, H, TSH), np.uint16)}
            _state["prep_ws"] = ws
        out = ws["u16"]

        def one(c):
            w = ws["f32"][c]
            np.copyto(w, a[c].T)
            v = w.view(np.uint32)
            t = ws["tmp"][c]
            np.right_shift(v, 16, out=t)
            np.bitwise_and(t, 1, out=t)
            np.add(t, 0x7FFF, out=t)
            np.add(v, t, out=v)
            np.right_shift(v, 16, out=v)
            out[c] = v  # u32 -> u16 narrowing copy

        list(_pool().map(one, range(NCORES)))
        return {"hs_sh": out.reshape(NCORES * H, TSH).view(ml_dtypes.bfloat16)}
    q = np.empty((NCORES, H, TSH), np.int8)
    s = np.empty((NCORES, 128, KCH), np.float32)

    def one(c):
        x = np.ascontiguousarray(a[c].T)  # [H, TSH]
        am = np.abs(x).max(axis=1)
        np.maximum(am, 1e-30, out=am)
        np.rint(x * (127.0 / am)[:, None], out=x)
        q[c] = x  # exact integers, cast is lossless
        # device wants scales as [ki, ko]; h = ko*128 + ki
        s[c] = (am * (1.0 / 127.0)).reshape(KCH, 128).T

    list(_pool().map(one, range(NCORES)))
    return {"hs_sh": q.reshape(NCORES * H, TSH),
            "hsc_sh": s.reshape(NCORES * 128, KCH)}


def _prep_consts(Wq, Wk, Wv, Wo, position_ids):
    bf16 = ml_dtypes.bfloat16
    inv = (1.0 / (ROPE_THETA ** (np.arange(0, HD, 2, dtype=np.float32) / HD)))
    pos = np.asarray(position_ids).reshape(T).astype(np.float32)
    fr = pos[None, :] * inv[:, None]  # [64, T]
    cos = np.cos(fr)
    sin = np.sin(fr)
    cs = np.concatenate([cos, cos, -sin, sin], axis=0).astype(bf16)  # [256, T]

    def wcat(Wfull, wd):
        Wfull = np.asarray(Wfull, dtype=np.float32)
        R = Wfull.shape[0]
        return np.ascontiguousarray(
            Wfull.reshape(R, NCORES, wd).transpose(1, 0, 2)).astype(
                bf16).reshape(NCORES * R, wd)

    return {
        "wq": wcat(Wq, DQ),
        "wk": wcat(Wk, HD),
        "wv": wcat(Wv, HD),
        "wo": wcat(Wo, DQ),
        "cs_sh": _shard0(cs, TSH),
    }


def _prep_inputs(hidden_states, Wq, Wk, Wv, Wo, position_ids):
    out = {"hs_sh": _prep_hs(hidden_states)}
    out.update(_prep_consts(Wq, Wk, Wv, Wo, position_ids))
    return out


def _get_runner():
    """Build the sharded jit once; reuse across kernel() calls."""
    if "runner" in _state:
        return _state["runner"]

    import jax
    import jax.numpy as jnp
    import concourse.mybir as mybir
    from concourse import bass2jax
    from jax.sharding import Mesh, PartitionSpec, NamedSharding
    from jax.experimental.shard_map import shard_map

    nc = _get_nc()
    bass2jax.install_neuronx_cc_hook()

    in_names = []
    out_names = []
    out_avals = []
    zero_shapes = []
    for alloc in nc.m.functions[0].allocations:
        if not isinstance(alloc, mybir.MemoryLocationSet):
            continue
        name = alloc.memorylocations[0].name
        if alloc.kind == "ExternalInput":
            if nc.partition_id_tensor is None or name != nc.partition_id_tensor.name:
                in_names.append(name)
        elif alloc.kind == "ExternalOutput":
            shape = tuple(alloc.tensor_shape)
            dtype = mybir.dt.np(alloc.dtype)
            out_names.append(name)
            out_avals.append(jax.core.ShapedArray(shape, dtype))
            zero_shapes.append(((NCORES * shape[0],) + shape[1:], dtype))

    n_params = len(in_names)
    n_outs = len(out_avals)
    all_in_names = list(in_names) + list(out_names)
    if nc.partition_id_tensor is not None:
        all_in_names.append(nc.partition_id_tensor.name)

    def _body(*args):
        operands = list(args)
        if nc.partition_id_tensor is not None:
            operands.append(bass2jax.partition_id_tensor())
        outs = bass2jax._bass_exec_p.bind(
            *operands,
            out_avals=tuple(out_avals),
            in_names=tuple(all_in_names),
            out_names=tuple(out_names),
            lowering_input_output_aliases=(),
            sim_require_finite=True,
            sim_require_nnan=True,
            nc=nc,
        )
        return tuple(outs)

    devices = jax.devices()[:NCORES]
    mesh = Mesh(np.asarray(devices), ("core",))
    in_specs = (PartitionSpec("core"),) * (n_params + n_outs)
    out_specs = (PartitionSpec("core"),) * n_outs
    donate = tuple(range(n_params, n_params + n_outs))
    sharded = jax.jit(
        shard_map(_body, mesh=mesh, in_specs=in_specs, out_specs=out_specs,
                  check_rep=False),
        donate_argnums=donate, keep_unused=True)

    sh = NamedSharding(mesh, PartitionSpec("core"))
    zeros_fn = jax.jit(
        lambda: tuple(jnp.zeros(s, d) for s, d in zero_shapes),
        out_shardings=tuple(sh for _ in zero_shapes))

    import os
    import time
    dbg = bool(os.environ.get("KERN_TIMING"))

    def run(cat_map):
        # values may be host numpy arrays (transferred now) or cached
        # device-resident jax arrays (no transfer)
        t0 = time.perf_counter()
        ins = [cat_map[name] for name in in_names]
        # donated output buffers: recycle the previous call's (fully
        # overwritten) device outputs; fall back to fresh device zeros
        zs = _state.pop("recycle", None)
        if zs is None:
            zs = zeros_fn()
        if dbg:
            th0 = time.perf_counter()
            import jax as _jax
            ins[0] = _jax.device_put(ins[0], sh)
            ins[0].block_until_ready()
            t1 = time.perf_counter()
            print(f"    [run] donbuf {th0-t0:.3f}s  hs H2D {t1-th0:.3f}s",
                  flush=True)
        out_arrs = sharded(*ins, *zs)
        _state["recycle"] = tuple(out_arrs)
        return {name: out_arrs[i] for i, name in enumerate(out_names)}

    _state["sharding"] = sh
    _state["runner"] = run
    return run


def kernel(hidden_states, Wq, Wk, Wv, Wo, attention_mask, position_ids):
    """Weights + RoPE tables are kept device-resident across calls, guarded by
    a full content comparison against stashed host copies (so a call with new
    weights re-uploads). Activations are prepped + transferred every call."""
    import jax
    import os
    import time

    dbg = bool(os.environ.get("KERN_TIMING"))
    t0 = time.perf_counter()
    run = _get_runner()
    key_arrays = [np.asarray(x) for x in (Wq, Wk, Wv, Wo, position_ids)]
    wc = _state.get("wcache")
    # fast path: same array objects as last call; else full content compare
    hit = wc is not None and (
        all(a is b for a, b in zip(key_arrays, wc["orig"]))
        or all(a.shape == b.shape and a.dtype == b.dtype and np.array_equal(a, b)
               for a, b in zip(key_arrays, wc["host"])))
    if dbg:
        t1 = time.perf_counter()
        print(f"    [kern] wcheck {t1-t0:.3f}s hit={hit}", flush=True)
    if not hit:
        consts = _prep_consts(Wq, Wk, Wv, Wo, position_ids)
        dev = {k: jax.device_put(v, _state["sharding"])
               for k, v in consts.items()}
        for v in dev.values():
            v.block_until_ready()
        wc = {"orig": key_arrays, "host": [a.copy() for a in key_arrays],
              "dev": dev}
        _state["wcache"] = wc

    t2 = time.perf_counter()
    cat = _prep_hs(hidden_states)
    cat.update(wc["dev"])
    if dbg:
        t3 = time.perf_counter()
        print(f"    [kern] hsprep {t3-t2:.3f}s", flush=True)
    devarrs = run(cat)
    t4 = time.perf_counter()
    # fetch every device shard concurrently (overlaps the per-array sync
    # round-trips and the dequant work with the D2H stream), dequantize and
    # transpose-assemble: out[t, fg] = i8[fg, t] * sc[fg, t//OG]; core c owns
    # output feature columns [c*DQ, (c+1)*DQ)
    out = np.empty((T, H), dtype=np.float32)

    def by_core(garr):
        m = {}
        for s in garr.addressable_shards:
            m[(s.index[0].start or 0) // DQ] = s.data
        return m

    i8_by_c = by_core(devarrs["outT_i8"])
    sc_by_c = by_core(devarrs["oscale"])

    def onec(c):
        scc = np.asarray(sc_by_c[c]).astype(np.float32) * (1.0 / 127.0)
        i8c = np.asarray(i8_by_c[c])  # [DQ, T] int8
        col = c * DQ
        for j in range(NOG):
            np.multiply(i8c[:, j * OG:(j + 1) * OG].T.astype(np.float32),
                        scc[:, j][None, :],
                        out=out[j * OG:(j + 1) * OG, col:col + DQ])

    list(_pool().map(onec, range(NCORES)))
    out = out.reshape(B, S, H)
    if dbg:
        t5 = time.perf_counter()
        print(f"    [kern] assemble {t5-t4:.3f}s", flush=True)
    return out
